# revision 4
# baseline (speedup 1.0000x reference)
"""TRN2 Bass kernel for FFQLinear: y = x @ ((q - zp) * scale) + bias.

x: [2, 2048, 4096] f32, q: [4096, 4096] int32 (values 0..255),
scale/zero_point: [1] f32, bias: [4096] f32 -> y: [2, 2048, 4096] f32.

Strategy (8 NeuronCores, M split 8 ways, dequantized weight replicated):
  - Host prep (once, per call): w = ((q - zp) * scale) in fp16
    [DIN, DOUT]; x shard transposed to mi-major [128, 4, 32, 128] fp16
    so the contraction dim lands on SBUF partitions with no on-device
    transpose and each m-tile chunk is a contiguous DMA; bias f32.
  - Per core device graph: double-buffered resident xT loaded as 4
    per-mi chunk DMAs (rep r+1's chunks land during rep r's tail);
    stream w in [128, 32, 1024] fp16 pair-panels (double buffered, 2KB
    DMA lines), each panel split into two half-DMAs; DMA issue order
    puts the first xT chunk and first w half ahead so the first PSUM
    group starts after ~1/8 of the fill. bias streamed per-panel as
    [128, 1024] f32 slices (8KB/partition, bufs=2) to fit xT x2 in
    SBUF (64+128+8+4 = 204KB of ~208 usable). Groups ordered
    sub-outer, mi-inner; per panel run 8 PSUM accumulation groups
    STRICTLY SEQUENTIALLY (32 back-to-back matmuls per group -
    interleaving groups across PSUM banks measured 2.8x slower on HW);
    epilogue is a single f32 bias add on the DVE, then DMA out.
"""
import numpy as np


def _ensure_paths():
    import sys
    try:
        import concourse  # noqa: F401
        return
    except ImportError:
        pass
    for p in ("/opt/trn_rl_repo", "/root/.axon_site/_ro/trn_rl_repo"):
        if p not in sys.path:
            sys.path.insert(0, p)
    import concourse  # noqa: F401


B, S, DIN, DOUT = 2, 2048, 4096, 4096
N_CORES = 8
M_SH = (B * S) // N_CORES        # 512 rows per core
P = 128
KO = DIN // P                    # 32 k-tiles
MT = M_SH // P                   # 4 m-tiles
NTILE = 512
NPAIR = 1024                     # w panel width (2 n-tiles)
NP = DOUT // NPAIR               # 4 w pair-panels


def _build(reps: int = 1):
    from contextlib import ExitStack
    import concourse.bass as bass
    import concourse.tile as tile
    from concourse import bacc, mybir
    from concourse.bass import ts

    f32 = mybir.dt.float32
    f16 = mybir.dt.float16

    nc = bacc.Bacc("TRN2", target_bir_lowering=False, debug=False)

    xts = nc.dram_tensor("xts", [P, MT, KO, P], f16, kind="ExternalInput")
    ws = nc.dram_tensor("ws", [DIN, DOUT], f16, kind="ExternalInput")
    biass = nc.dram_tensor("biass", [DOUT], f32, kind="ExternalInput")
    ys = nc.dram_tensor("ys", [M_SH, DOUT], f32, kind="ExternalOutput")

    ws_t = ws.rearrange("(ko p) n -> p ko n", p=P)

    with tile.TileContext(nc) as tc, ExitStack() as ctx:
        xt_pool = ctx.enter_context(tc.tile_pool(name="xt_pool", bufs=2))
        w_pool = ctx.enter_context(tc.tile_pool(name="w_pool", bufs=2))
        b_pool = ctx.enter_context(tc.tile_pool(name="b_pool", bufs=2))
        y_pool = ctx.enter_context(tc.tile_pool(name="y_pool", bufs=2))
        psum = ctx.enter_context(
            tc.tile_pool(name="psum", bufs=8, space="PSUM"))

        def body():
            # resident transposed-x panel: xT[p, mi, ki, mc] =
            # x[mi*128+mc, ki*128+p], loaded per-mi so the first group
            # only waits on one chunk + half a w panel
            xT = xt_pool.tile([P, MT, KO, P], f16, tag="xT")
            wp0 = w_pool.tile([P, KO, NPAIR], f16, tag="wp")
            nc.sync.dma_start(xT[:, 0], xts[:, 0])
            nc.sync.dma_start(wp0[:, :, 0:NTILE], ws_t[:, :, 0:NTILE])
            for mi in range(1, MT):
                nc.sync.dma_start(xT[:, mi], xts[:, mi])
            nc.sync.dma_start(wp0[:, :, NTILE:NPAIR],
                              ws_t[:, :, NTILE:NPAIR])

            for np_ in range(NP):
                if np_ == 0:
                    wp = wp0
                else:
                    wp = w_pool.tile([P, KO, NPAIR], f16, tag="wp")
                    base = np_ * NPAIR
                    nc.sync.dma_start(wp[:, :, 0:NTILE],
                                      ws_t[:, :, base:base + NTILE])
                    nc.sync.dma_start(wp[:, :, NTILE:NPAIR],
                                      ws_t[:, :, base + NTILE:base + NPAIR])
                bias_t = b_pool.tile([P, NPAIR], f32, tag="bias")
                nc.sync.dma_start(
                    bias_t[:],
                    biass[ts(np_, NPAIR)].partition_broadcast(P))
                for sub in range(NPAIR // NTILE):
                    for mi in range(MT):
                        acc = psum.tile([P, NTILE], f32, tag="acc",
                                        name=f"acc_{np_}_{sub}_{mi}")
                        for ki in range(KO):
                            nc.tensor.matmul(
                                acc[:], lhsT=xT[:, mi, ki],
                                rhs=wp[:, ki, ts(sub, NTILE)],
                                start=(ki == 0), stop=(ki == KO - 1))
                        y = y_pool.tile([P, NTILE], f32, tag="y")
                        nc.vector.tensor_tensor(
                            y[:], acc[:], bias_t[:, ts(sub, NTILE)],
                            mybir.AluOpType.add)
                        ncol = np_ * NPAIR + sub * NTILE
                        nc.sync.dma_start(
                            ys[ts(mi, P), ncol:ncol + NTILE], y[:])

        if reps == 1:
            body()
        else:
            with tc.For_i(0, reps, 1):
                body()

    nc.compile()
    return nc


def prep_inputs(x: np.ndarray, q_int_weight: np.ndarray, scale: np.ndarray,
                zero_point: np.ndarray, bias: np.ndarray):
    """Host-side shard prep: dequantize w to fp16, transpose+cast+pack x
    shards mi-major."""
    scale_f = np.float32(np.asarray(scale).reshape(-1)[0])
    zp_f = np.float32(np.asarray(zero_point).reshape(-1)[0])
    w16 = ((q_int_weight.astype(np.float32) - zp_f) * scale_f).astype(
        np.float16)
    xf = x.reshape(B * S, DIN)
    bf = bias.astype(np.float32)
    in_maps = []
    for c in range(N_CORES):
        xs = xf[c * M_SH:(c + 1) * M_SH].astype(np.float16)
        # [m=(mi mc), k=(ki p)] -> [p, mi, ki, mc]
        xt = np.ascontiguousarray(
            xs.T.reshape(KO, P, MT, P).transpose(1, 2, 0, 3))
        in_maps.append({"xts": xt, "ws": w16, "biass": bf})
    return in_maps


def kernel(x: np.ndarray, q_int_weight: np.ndarray, scale: np.ndarray,
           zero_point: np.ndarray, bias: np.ndarray) -> np.ndarray:
    _ensure_paths()
    from concourse.bass_utils import run_bass_kernel_spmd

    nc = _build()
    in_maps = prep_inputs(x, q_int_weight, scale, zero_point, bias)
    res = run_bass_kernel_spmd(nc, in_maps, core_ids=list(range(N_CORES)))

    y = np.empty((B * S, DOUT), np.float32)
    for c in range(N_CORES):
        y[c * M_SH:(c + 1) * M_SH] = res.results[c]["ys"]
    return y.reshape(B, S, DOUT)
